# revision 44
# baseline (speedup 1.0000x reference)
"""Causal self-attention (B=1, T=4096, D=1024, H=16, HD=64) on 8 trn2 NeuronCores.

Sharding: tensor-parallel over heads (2 heads per core) for QKV + attention;
pipelined per-block-pair AllToAll re-shards to row-parallel for the output
projection (each core projects a 128-row slice of each 1024-row block pair).

Matmul layout notes (PE computes out = lhsT.T @ rhs, contraction on partitions):
 - host feeds x transposed (xT [D, T]) so QKV needs no on-chip transposes.
 - S^T tiles [tk, tq] are computed (not S) so the PV matmul can consume
   exp(S^T) directly as the moving operand with V in natural [tk, hd] layout.
 - S tiles are chunk-major with BOTH heads in one [128, 1024] tile
   (head0 at cols 0:512, head1 at 512:1024). The heads' QK matmuls use PE
   row tiles (0,0)/(64,0) and share one PSUM tile, so one exp releases
   both and the pair co-issues on the PE's 64-row tile mode.
 - a ones-column appended to V makes row 64 of the PV accumulator the
   softmax denominator (no extra reduction pass).
 - causal masking: QK/exp/PV streams are trimmed to the causally valid
   column range; the 128-wide diagonal band is zeroed in p by a DVE
   multiply with a precomputed triangle mask (no PE mask matmuls).
 - the whole attention stream runs as one flat software pipeline across
   block boundaries (exp(i) | S(i+1) | PV(i-1)) so the PE never drains at
   a block edge (keeps the PE p-state ramped).
 - softmax max-subtraction is skipped: scores are ~N(0,1) (|s| < ~10), and
   a constant shift cancels exactly in softmax, so exp is safe in fp32.
"""

import math
import sys
from contextlib import ExitStack

sys.path.insert(0, "/opt/trn_rl_repo")

import ml_dtypes
import numpy as np

import concourse.bass as bass  # noqa: F401  (bass types used via tile/bacc)
import concourse.mybir as mybir
import concourse.tile as tile
from concourse import bacc
from concourse.bass_utils import run_bass_kernel_spmd

B, T, D, H, HD = 1, 4096, 1024, 16, 64
NCORES = 8
HPC = H // NCORES          # heads per core = 2
E = HPC * HD               # per-core head width = 128
TQ = 512                   # tq block width
NB = T // TQ               # 8 tq blocks
CK = 128                   # tk chunk (partition dim of S^T tiles)
KD = D // 128              # 8 contraction chunks over D
NV = T // CK               # 32 tk chunks total
VW = HD + 1                # V tile width incl. ones column = 65
NP = NB // 2               # block pairs exchanged together = 4

BF16 = mybir.dt.bfloat16
F32 = mybir.dt.float32
NPBF16 = ml_dtypes.bfloat16

_CACHE = {}


def _build():
    nc = bacc.Bacc("TRN2", target_bir_lowering=False, debug=False, num_devices=NCORES)
    xT = nc.dram_tensor("xT", [D, T], BF16, kind="ExternalInput").ap()
    wqT = nc.dram_tensor("wqT", [D, E], BF16, kind="ExternalInput").ap()
    wkT = nc.dram_tensor("wkT", [D, E], BF16, kind="ExternalInput").ap()
    wvT = nc.dram_tensor("wvT", [D, E], BF16, kind="ExternalInput").ap()
    wpT = nc.dram_tensor("wpT", [D, D], BF16, kind="ExternalInput").ap()
    tri = nc.dram_tensor("tri", [128, 128], BF16, kind="ExternalInput").ap()
    out = nc.dram_tensor("out", [NP * 128, D], F32, kind="ExternalOutput").ap()

    with tile.TileContext(nc) as tc, ExitStack() as ctx:
        sing = ctx.enter_context(tc.tile_pool(name="sing", bufs=1))
        pwork = ctx.enter_context(tc.tile_pool(name="pwork", bufs=6))
        ynp = ctx.enter_context(tc.tile_pool(name="ynp", bufs=4))
        ybp = ctx.enter_context(tc.tile_pool(name="ybp", bufs=2))
        osb = ctx.enter_context(tc.tile_pool(name="osb", bufs=2))
        # PSUM (8 banks): psS 3 x [128,1024] (2 banks each) = 6 for S tiles,
        # QKV accumulators and proj; psY 2 x [65,512] (1 bank each) = 2.
        psS = ctx.enter_context(tc.tile_pool(name="psS", bufs=3, space="PSUM"))
        psY = ctx.enter_context(tc.tile_pool(name="psY", bufs=2, space="PSUM"))
        dram = ctx.enter_context(tc.tile_pool(name="dram", bufs=1, space="DRAM"))

        # ---- resident SBUF tensors -------------------------------------
        xT_sb = sing.tile([128, KD * T], BF16)      # d-chunk kc at cols [kc*T, (kc+1)*T)
        wq_sb = sing.tile([128, KD * E], BF16)
        wk_sb = sing.tile([128, KD * E], BF16)
        wv_sb = sing.tile([128, KD * E], BF16)
        wp_sb = sing.tile([128, KD * D], BF16)
        tri_sb = sing.tile([128, 128], BF16)        # tri[k,j] = 1{j>=k}
        qT_sb = sing.tile([128, T], BF16)           # rows 0:64 head0, 64:128 head1
        kT_sb = sing.tile([128, T], BF16)
        v0_sb = sing.tile([128, NV * VW], BF16)     # V head0 + ones col per chunk
        v1_sb = sing.tile([128, NV * VW], BF16)
        y_sb = sing.tile([128, T], F32)             # unnormalized y^T (both heads)
        dsp_sb = sing.tile([128, 4 * TQ], BF16)     # den rows at partitions {0,32,64,96}
        onesp_sb = sing.tile([128, 128], BF16)
        zb_sb = sing.tile([128, 1], F32)            # zero bias for activations

        nc.vector.memset(zb_sb[:], 0.0)
        nc.vector.memset(onesp_sb[:], 1.0)
        nc.vector.memset(
            v0_sb[:].rearrange("p (c w) -> p c w", w=VW)[:, :, HD : HD + 1], 1.0
        )
        nc.vector.memset(
            v1_sb[:].rearrange("p (c w) -> p c w", w=VW)[:, :, HD : HD + 1], 1.0
        )

        # ---- input DMAs (weights first; xT t-slice-major) ---------------
        def load_w(w_sb, w_dram):
            nc.sync.dma_start(
                out=w_sb[:].rearrange("p (c e) -> p c e", c=KD),
                in_=w_dram.rearrange("(c p) e -> p c e", p=128),
            )

        def load_xt(tb):
            # two DMA calls so the transfer spreads over two queues
            for h in range(2):
                c0, c1 = h * (KD // 2), (h + 1) * (KD // 2)
                nc.sync.dma_start(
                    out=xT_sb[:].rearrange("p (c t) -> p c t", c=KD)[
                        :, c0:c1, tb * TQ : (tb + 1) * TQ
                    ],
                    in_=xT[c0 * 128 : c1 * 128, tb * TQ : (tb + 1) * TQ].rearrange(
                        "(c p) t -> p c t", p=128
                    ),
                )

        load_xt(0)
        load_w(wq_sb, wqT)
        load_w(wk_sb, wkT)
        load_w(wv_sb, wvT)
        nc.scalar.dma_start(out=tri_sb[:], in_=tri)
        load_xt(1)

        def load_wp():
            nc.scalar.dma_start(
                out=wp_sb[:].rearrange("p (c e) -> p c e", c=KD),
                in_=wpT.rearrange("(c p) e -> p c e", p=128),
            )

        # ---- QKV helpers (emitted lazily as PE filler) -------------------
        def emit_q(tb):
            ts = tb * TQ
            psq = psS.tile([128, 1024], F32, tag="ps", name=f"psq{tb}")
            for kc in range(KD):
                nc.tensor.matmul(
                    out=psq[:, 0:TQ],
                    lhsT=wq_sb[:, kc * E : (kc + 1) * E],
                    rhs=xT_sb[:, kc * T + ts : kc * T + ts + TQ],
                    start=(kc == 0),
                    stop=(kc == KD - 1),
                )
            nc.vector.tensor_copy(out=qT_sb[:, ts : ts + TQ], in_=psq[:, 0:TQ])

        def emit_k(tb):
            ts = tb * TQ
            psk = psS.tile([128, 1024], F32, tag="ps", name=f"psk{tb}")
            for kc in range(KD):
                nc.tensor.matmul(
                    out=psk[:, 0:TQ],
                    lhsT=wk_sb[:, kc * E : (kc + 1) * E],
                    rhs=xT_sb[:, kc * T + ts : kc * T + ts + TQ],
                    start=(kc == 0),
                    stop=(kc == KD - 1),
                )
            nc.vector.tensor_copy(out=kT_sb[:, ts : ts + TQ], in_=psk[:, 0:TQ])

        def emit_v(ci):
            psv = psS.tile([128, 1024], F32, tag="ps", name=f"psv{ci}")
            for kc in range(KD):
                nc.tensor.matmul(
                    out=psv[:, 0:E],
                    lhsT=xT_sb[:, kc * T + ci * CK : kc * T + (ci + 1) * CK],
                    rhs=wv_sb[:, kc * E : (kc + 1) * E],
                    start=(kc == 0),
                    stop=(kc == KD - 1),
                )
            nc.vector.tensor_copy(
                out=v0_sb[:, ci * VW : ci * VW + HD], in_=psv[:, 0:HD]
            )
            nc.vector.tensor_copy(
                out=v1_sb[:, ci * VW : ci * VW + HD], in_=psv[:, HD:E]
            )

        # ---- exchange buffers: pairs 0-2 (shard s = 128 rows of the pair);
        # the last pair goes as two per-block exchanges (shard s = 64 rows)
        # so block 6's hides under block 7 and only block 7's is exposed.
        send_t = [dram.tile([NCORES, 128, 128], BF16, name=f"snd{k}") for k in range(NP - 1)]
        recv_t = [dram.tile([NCORES, 128, 128], BF16, name=f"rcv{k}") for k in range(NP - 1)]
        send_b = {b: dram.tile([NCORES, 128, 64], BF16, name=f"sndb{b}") for b in (6, 7)}
        recv_b = {b: dram.tile([NCORES, 128, 64], BF16, name=f"rcvb{b}") for b in (6, 7)}
        wup_s = dram.tile([NCORES, 128, 8], BF16, name="wups")
        wup_r = dram.tile([NCORES, 128, 8], BF16, name="wupr")

        def _a2a(src, dst):
            nc.gpsimd.collective_compute(
                "AllToAll",
                mybir.AluOpType.bypass,
                replica_groups=[list(range(NCORES))],
                ins=[src[:].opt()],
                outs=[dst[:].opt()],
            )

        def emit_a2a(k):
            _a2a(send_t[k], recv_t[k])

        # warmup collective: absorbs the CC entry-barrier setup cost early
        _a2a(wup_s, wup_r)

        def emit_norm_send(b, ylast=None, tail=False):
            # GpSimd broadcasts the denominator rows across partitions, then
            # DVE takes 1/x and one multiply; ship the block into its send
            # buffer (pair shards 0-3/4-7, or 8x64-row shards for blocks 6/7).
            rbb = psS.tile([128, TQ], F32, tag="ps", name=f"rbb{b}")
            for h in range(2):
                i = 2 * b + h
                pr = (i % 4) * 32
                cr = (i // 4) * TQ
                nc.tensor.matmul(
                    out=rbb[h * HD : (h + 1) * HD, :],
                    lhsT=onesp_sb[pr : pr + 1, 0:HD],
                    rhs=dsp_sb[pr : pr + 1, cr : cr + TQ],
                    start=True,
                    stop=True,
                    tile_position=(pr, h * HD),
                )
            rq = ynp.tile([128, TQ], F32, tag="rq", name=f"rq{b}")
            nc.vector.reciprocal_approx_fast(out=rq[:], in_=rbb[:])
            yn = ynp.tile([128, TQ], BF16, tag="yn", name=f"yn{b}")
            if ylast is None:
                nc.vector.tensor_mul(yn[:], y_sb[:, b * TQ : (b + 1) * TQ], rq[:])
            else:
                # tail block: normalize straight from the PSUM accumulators,
                # heads on different engines so the muls overlap
                y0t, y1t = ylast
                nc.vector.tensor_mul(yn[0:HD, :], y0t[0:HD, :], rq[0:HD, :])
                nc.vector.tensor_mul(yn[HD:128, :], y1t[0:HD, :], rq[HD:128, :])
            if b >= NB - 2:
                nc.sync.dma_start(
                    out=send_b[b][:].rearrange("s p q -> p s q"),
                    in_=yn[:].rearrange("p (s q) -> p s q", s=8),
                )
            else:
                k = b // 2
                s0 = (b % 2) * 4
                nc.sync.dma_start(
                    out=send_t[k][:].rearrange("s p q -> p s q")[:, s0 : s0 + 4, :],
                    in_=yn[:].rearrange("p (s q) -> p s q", s=4),
                )

        ybtiles = {}

        def emit_recv(k):
            # recv shard j = head-dims d in [128j, 128j+128) for my 128 rows
            yb = ybp.tile([128, NCORES * 128], BF16, tag="yb", name=f"yb{k}")
            nc.sync.dma_start(
                out=yb[:].rearrange("p (s q) -> p s q", s=NCORES),
                in_=recv_t[k][:].rearrange("s p q -> p s q"),
            )
            ybtiles[k] = yb

        def emit_projmm(k):
            yb = ybtiles[k]
            po = psS.tile([128, 1024], F32, tag="ps", name=f"po{k}")
            for nh in range(2):
                for j in range(NCORES):
                    nc.tensor.matmul(
                        out=po[:, nh * 512 : (nh + 1) * 512],
                        lhsT=yb[:, j * 128 : (j + 1) * 128],
                        rhs=wp_sb[:, j * D + nh * 512 : j * D + (nh + 1) * 512],
                        start=(j == 0),
                        stop=(j == NCORES - 1),
                    )
            o_sb = osb.tile([128, 1024], F32, tag="o", name=f"osb{k}")
            nc.vector.tensor_copy(out=o_sb[:], in_=po[:])
            nc.sync.dma_start(out=out[k * 128 : (k + 1) * 128, :], in_=o_sb[:])

        # last pair: per-block recv + half-height (M=64) projections so the
        # block-6 half runs hidden under block 7's attention.
        def emit_recv_b(b):
            yb = ybp.tile([128, NCORES * 64], BF16, tag="ybh", name=f"ybb{b}")
            nc.sync.dma_start(
                out=yb[:].rearrange("p (s q) -> p s q", s=NCORES),
                in_=recv_b[b][:].rearrange("s p q -> p s q"),
            )
            ybtiles[b + 100] = yb

        o3_sb = sing.tile([128, 1024], F32)

        def emit_projmm_b(b):
            yb = ybtiles[b + 100]
            po = psS.tile([128, 1024], F32, tag="ps", name=f"pob{b}")
            r0 = (b - 6) * 64
            for nh in range(2):
                for j in range(NCORES):
                    nc.tensor.matmul(
                        out=po[r0 : r0 + 64, nh * 512 : (nh + 1) * 512],
                        lhsT=yb[:, j * 64 : (j + 1) * 64],
                        rhs=wp_sb[:, j * D + nh * 512 : j * D + (nh + 1) * 512],
                        start=(j == 0),
                        stop=(j == NCORES - 1),
                    )
            nc.vector.tensor_copy(out=o3_sb[r0 : r0 + 64, :], in_=po[r0 : r0 + 64, :])

        # ---- flat attention pipeline -------------------------------------
        sched = [(b, sc) for b in range(NB) for sc in range(2 * (b + 1))]
        ytiles = {}

        def get_y(b):
            if b not in ytiles:
                ytiles[b] = (
                    psY.tile([VW, TQ], F32, tag="py", name=f"y0_{b}"),
                    psY.tile([VW, TQ], F32, tag="py", name=f"y1_{b}"),
                )
            return ytiles[b]

        def emit_s(b, sc):
            # two chunk tiles, each holding both heads (h0 cols 0:512,
            # h1 cols 512:1024); heads co-issue on PE row tiles.
            ts = b * TQ
            tiles = []
            for j in range(2):
                ci = 2 * sc + j
                off = ci * CK - ts
                qlo = max(off, 0)
                s = psS.tile([128, 1024], F32, tag="ps", name=f"s{b}_{sc}_{j}")
                nc.tensor.matmul(
                    out=s[:, qlo:TQ],
                    lhsT=kT_sb[0:HD, ci * CK : (ci + 1) * CK],
                    rhs=qT_sb[0:HD, ts + qlo : ts + TQ],
                    start=True,
                    stop=True,
                    tile_position=(0, 0),
                )
                nc.tensor.matmul(
                    out=s[:, TQ + qlo : 2 * TQ],
                    lhsT=kT_sb[HD:128, ci * CK : (ci + 1) * CK],
                    rhs=qT_sb[HD:128, ts + qlo : ts + TQ],
                    start=True,
                    stop=True,
                    tile_position=(64, 0),
                )
                tiles.append(s)
            return tiles

        def emit_p(b, sc, stiles):
            ts = b * TQ
            ptiles = []
            for j in range(2):
                ci = 2 * sc + j
                off = ci * CK - ts
                qlo = max(off, 0)
                s = stiles[j]
                p = pwork.tile([128, 1024], BF16, tag="pt", name=f"p{b}_{sc}_{j}")
                if qlo == 0:
                    ranges = [(0, 1024)]
                elif qlo <= CK:
                    # cols [TQ, TQ+qlo) hold garbage but are never consumed;
                    # one instr beats two for a small overlap
                    ranges = [(qlo, 2 * TQ)]
                else:
                    ranges = [(qlo, TQ), (TQ + qlo, 2 * TQ)]
                for (lo, hi) in ranges:
                    nc.scalar.activation(
                        out=p[:, lo:hi], in_=s[:, lo:hi],
                        func=mybir.ActivationFunctionType.Exp, bias=zb_sb[:],
                    )
                if off >= 0:
                    nc.vector.tensor_mul(
                        p[:, off : off + CK], p[:, off : off + CK], tri_sb[:]
                    )
                    nc.vector.tensor_mul(
                        p[:, TQ + off : TQ + off + CK],
                        p[:, TQ + off : TQ + off + CK],
                        tri_sb[:],
                    )
                ptiles.append(p)
            return ptiles

        def emit_pv(b, sc, ptiles):
            ts = b * TQ
            nchunks = 4 * (b + 1)
            y0t, y1t = get_y(b)
            for j in range(2):
                ci = 2 * sc + j
                off = ci * CK - ts
                qlo = max(off, 0)
                p = ptiles[j]
                nc.tensor.matmul(
                    out=y0t[:, qlo:TQ],
                    lhsT=v0_sb[:, ci * VW : (ci + 1) * VW],
                    rhs=p[:, qlo:TQ],
                    start=(ci == 0),
                    stop=(ci == nchunks - 1),
                )
                nc.tensor.matmul(
                    out=y1t[:, qlo:TQ],
                    lhsT=v1_sb[:, ci * VW : (ci + 1) * VW],
                    rhs=p[:, TQ + qlo : 2 * TQ],
                    start=(ci == 0),
                    stop=(ci == nchunks - 1),
                )

        def emit_block_end(b):
            # stash unnormalized y + denominator rows; block 7 keeps its
            # PSUM accumulators live for the tail's direct normalization.
            y0t, y1t = ytiles[b]
            for h, yy in ((0, y0t), (1, y1t)):
                i = 2 * b + h
                dst = dsp_sb[
                    (i % 4) * 32 : (i % 4) * 32 + 1,
                    (i // 4) * TQ : (i // 4 + 1) * TQ,
                ]
                if b == NB - 1 and h == 1:
                    nc.scalar.copy(out=dst, in_=yy[HD : HD + 1, :])
                else:
                    nc.vector.tensor_copy(out=dst, in_=yy[HD : HD + 1, :])
            if b < NB - 1:
                ts = b * TQ
                nc.vector.tensor_copy(out=y_sb[0:HD, ts : ts + TQ], in_=y0t[0:HD, :])
                nc.vector.tensor_copy(out=y_sb[HD:128, ts : ts + TQ], in_=y1t[0:HD, :])

        # pre/post hooks per (b, sc) slot
        hooks = {}

        def add_hook(b, sc, fn):
            hooks.setdefault((b, sc), []).append(fn)

        for b in range(1, NB):
            add_hook(b, 0, lambda bb=b: emit_norm_send(bb - 1))
            if b % 2 == 0 and b < NB - 1:
                add_hook(b, 0, lambda kk=b // 2 - 1: emit_a2a(kk))
        add_hook(NB - 1, 0, lambda: _a2a(send_b[6], recv_b[6]))
        add_hook(NB - 1, 10, lambda: emit_recv_b(6))
        add_hook(NB - 1, 12, lambda: emit_projmm_b(6))
        add_hook(
            NB - 1, 14,
            lambda: nc.sync.dma_start(
                out=out[(NP - 1) * 128 : (NP - 1) * 128 + 64, :], in_=o3_sb[0:64, :]
            ),
        )
        add_hook(3, 2, lambda: emit_recv(0))
        add_hook(4, 2, lambda: emit_projmm(0))
        add_hook(5, 2, lambda: emit_recv(1))
        add_hook(6, 2, lambda: emit_projmm(1))
        add_hook(7, 2, lambda: emit_recv(2))
        add_hook(7, 8, lambda: emit_projmm(2))
        # deferred input loads: xT block b arrives well ahead of first use
        for b in range(2, NB):
            add_hook(max(b - 3, 0), 1, lambda tb=b: load_xt(tb))
        add_hook(1, 1, load_wp)

        # QKV filler items per block, clustered into two groups per block to
        # minimize PE mode-switch drains at filler boundaries
        add_hook(0, 0, lambda: emit_q(1))
        add_hook(0, 1, lambda: emit_k(1))
        for c in range(4, 8):
            add_hook(0, 1, lambda ci=c: emit_v(ci))
        for b in range(1, NB - 1):
            # q/k feed the NEXT blocks' S tiles: emit in the early group, well
            # before the flat pipeline's s_next reaches them
            items = [lambda tb=b + 1: emit_q(tb)]
            if b % 2 == 1 and b + 2 < NB:
                items.append(lambda tb=b + 1: emit_k(tb))
                items.append(lambda tb=b + 2: emit_k(tb))
            items += [lambda ci=c: emit_v(ci) for c in range(4 * b + 4, 4 * b + 8)]
            nsc = 2 * (b + 1)
            for idx, fn in enumerate(items):
                add_hook(b, 1 + (idx * (nsc - 2)) // len(items), fn)

        # prologue
        emit_q(0)
        emit_k(0)
        for ci in range(4):
            emit_v(ci)

        s_next = emit_s(*sched[0])
        p_prev = None
        prev_item = None
        for i, (b, sc) in enumerate(sched):
            cur_s = s_next
            p_cur = emit_p(b, sc, cur_s)
            if i + 1 < len(sched):
                s_next = emit_s(*sched[i + 1])
            if prev_item is not None:
                pb, psc = prev_item
                emit_pv(pb, psc, p_prev)
                if psc == 2 * (pb + 1) - 1:
                    emit_block_end(pb)
            for fn in hooks.get((b, sc), []):
                fn()
            p_prev, prev_item = p_cur, (b, sc)
        emit_pv(*prev_item, p_prev)
        emit_block_end(NB - 1)

        # ---- tail: last block's norm, exchange, projection ---------------
        emit_norm_send(NB - 1, ylast=ytiles[NB - 1])
        _a2a(send_b[7], recv_b[7])
        emit_recv_b(7)
        emit_projmm_b(7)
        nc.sync.dma_start(
            out=out[(NP - 1) * 128 + 64 : NP * 128, :], in_=o3_sb[64:128, :]
        )

    nc.compile()
    return nc


def _inputs(x, w_attn, w_proj):
    x = np.asarray(x, dtype=np.float32).reshape(T, D)
    w_attn = np.asarray(w_attn, dtype=np.float32)
    w_proj = np.asarray(w_proj, dtype=np.float32)

    xT_np = np.ascontiguousarray(x.T).astype(NPBF16)
    wpT_np = np.ascontiguousarray(w_proj.T).astype(NPBF16)
    scale = 1.0 / math.sqrt(HD)
    tri_np = np.triu(np.ones((128, 128), dtype=np.float32)).astype(NPBF16)

    in_maps = []
    for core in range(NCORES):
        r0 = core * E
        in_maps.append(
            {
                "xT": xT_np,
                "wqT": np.ascontiguousarray((w_attn[r0 : r0 + E, :] * scale).T).astype(
                    NPBF16
                ),
                "wkT": np.ascontiguousarray(w_attn[D + r0 : D + r0 + E, :].T).astype(
                    NPBF16
                ),
                "wvT": np.ascontiguousarray(
                    w_attn[2 * D + r0 : 2 * D + r0 + E, :].T
                ).astype(NPBF16),
                "wpT": wpT_np,
                "tri": tri_np,
            }
        )
    return in_maps


def kernel(x, w_attn, w_proj, _trace=False):
    if "nc" not in _CACHE:
        _CACHE["nc"] = _build()
    nc = _CACHE["nc"]
    in_maps = _inputs(x, w_attn, w_proj)
    res = run_bass_kernel_spmd(
        nc, in_maps, core_ids=list(range(NCORES)), trace=_trace
    )
    _CACHE["last_result"] = res
    full = np.empty((T, D), dtype=np.float32)
    for c in range(NCORES):
        o = res.results[c]["out"]  # [NP*128, D]
        for k in range(NP - 1):
            blk = 2 * k + (1 if c >= 4 else 0)
            q0 = (c % 4) * 128
            full[blk * TQ + q0 : blk * TQ + q0 + 128, :] = o[k * 128 : (k + 1) * 128, :]
        # last pair went as two per-block 64-row exchanges
        k = NP - 1
        q0 = c * 64
        full[6 * TQ + q0 : 6 * TQ + q0 + 64, :] = o[k * 128 : k * 128 + 64, :]
        full[7 * TQ + q0 : 7 * TQ + q0 + 64, :] = o[k * 128 + 64 : (k + 1) * 128, :]
    return full.reshape(B, T, D).astype(np.float32)


# revision 45
# speedup vs baseline: 1.0172x; 1.0172x over previous
"""Causal self-attention (B=1, T=4096, D=1024, H=16, HD=64) on 8 trn2 NeuronCores.

Sharding: tensor-parallel over heads (2 heads per core) for QKV + attention;
pipelined per-block-pair AllToAll re-shards to row-parallel for the output
projection (each core projects a 128-row slice of each 1024-row block pair).

Matmul layout notes (PE computes out = lhsT.T @ rhs, contraction on partitions):
 - host feeds x transposed (xT [D, T]) so QKV needs no on-chip transposes.
 - S^T tiles [tk, tq] are computed (not S) so the PV matmul can consume
   exp(S^T) directly as the moving operand with V in natural [tk, hd] layout.
 - S tiles are chunk-major with BOTH heads in one [128, 1024] tile
   (head0 at cols 0:512, head1 at 512:1024). The heads' QK matmuls use PE
   row tiles (0,0)/(64,0) and share one PSUM tile, so one exp releases
   both and the pair co-issues on the PE's 64-row tile mode.
 - a ones-column appended to V makes row 64 of the PV accumulator the
   softmax denominator (no extra reduction pass).
 - causal masking: QK/exp/PV streams are trimmed to the causally valid
   column range; the 128-wide diagonal band is zeroed in p by a DVE
   multiply with a precomputed triangle mask (no PE mask matmuls).
 - the whole attention stream runs as one flat software pipeline across
   block boundaries (exp(i) | S(i+1) | PV(i-1)) so the PE never drains at
   a block edge (keeps the PE p-state ramped).
 - softmax max-subtraction is skipped: scores are ~N(0,1) (|s| < ~10), and
   a constant shift cancels exactly in softmax, so exp is safe in fp32.
"""

import math
import sys
from contextlib import ExitStack

sys.path.insert(0, "/opt/trn_rl_repo")

import ml_dtypes
import numpy as np

import concourse.bass as bass  # noqa: F401  (bass types used via tile/bacc)
import concourse.mybir as mybir
import concourse.tile as tile
from concourse import bacc
from concourse.bass_utils import run_bass_kernel_spmd

B, T, D, H, HD = 1, 4096, 1024, 16, 64
NCORES = 8
HPC = H // NCORES          # heads per core = 2
E = HPC * HD               # per-core head width = 128
TQ = 512                   # tq block width
NB = T // TQ               # 8 tq blocks
CK = 128                   # tk chunk (partition dim of S^T tiles)
KD = D // 128              # 8 contraction chunks over D
NV = T // CK               # 32 tk chunks total
VW = HD + 1                # V tile width incl. ones column = 65
NP = NB // 2               # block pairs exchanged together = 4

BF16 = mybir.dt.bfloat16
F32 = mybir.dt.float32
NPBF16 = ml_dtypes.bfloat16

_CACHE = {}


def _build():
    nc = bacc.Bacc("TRN2", target_bir_lowering=False, debug=False, num_devices=NCORES)
    xT = nc.dram_tensor("xT", [D, T], BF16, kind="ExternalInput").ap()
    wqT = nc.dram_tensor("wqT", [D, E], BF16, kind="ExternalInput").ap()
    wkT = nc.dram_tensor("wkT", [D, E], BF16, kind="ExternalInput").ap()
    wvT = nc.dram_tensor("wvT", [D, E], BF16, kind="ExternalInput").ap()
    wpT = nc.dram_tensor("wpT", [D, D], BF16, kind="ExternalInput").ap()
    tri = nc.dram_tensor("tri", [128, 128], BF16, kind="ExternalInput").ap()
    out = nc.dram_tensor("out", [NP * 128, D], F32, kind="ExternalOutput").ap()

    with tile.TileContext(nc) as tc, ExitStack() as ctx:
        sing = ctx.enter_context(tc.tile_pool(name="sing", bufs=1))
        pwork = ctx.enter_context(tc.tile_pool(name="pwork", bufs=6))
        ynp = ctx.enter_context(tc.tile_pool(name="ynp", bufs=4))
        ybp = ctx.enter_context(tc.tile_pool(name="ybp", bufs=2))
        osb = ctx.enter_context(tc.tile_pool(name="osb", bufs=2))
        # PSUM (8 banks): psS 3 x [128,1024] (2 banks each) = 6 for S tiles,
        # QKV accumulators and proj; psY 2 x [65,512] (1 bank each) = 2.
        psS = ctx.enter_context(tc.tile_pool(name="psS", bufs=3, space="PSUM"))
        psY = ctx.enter_context(tc.tile_pool(name="psY", bufs=2, space="PSUM"))
        dram = ctx.enter_context(tc.tile_pool(name="dram", bufs=1, space="DRAM"))

        # ---- resident SBUF tensors -------------------------------------
        xT_sb = sing.tile([128, KD * T], BF16)      # d-chunk kc at cols [kc*T, (kc+1)*T)
        wq_sb = sing.tile([128, KD * E], BF16)
        wk_sb = sing.tile([128, KD * E], BF16)
        wv_sb = sing.tile([128, KD * E], BF16)
        wp_sb = sing.tile([128, KD * D], BF16)
        tri_sb = sing.tile([128, 128], BF16)        # tri[k,j] = 1{j>=k}
        qT_sb = sing.tile([128, T], BF16)           # rows 0:64 head0, 64:128 head1
        kT_sb = sing.tile([128, T], BF16)
        v0_sb = sing.tile([128, NV * VW], BF16)     # V head0 + ones col per chunk
        v1_sb = sing.tile([128, NV * VW], BF16)
        y_sb = sing.tile([128, T], F32)             # unnormalized y^T (both heads)
        dsp_sb = sing.tile([128, 4 * TQ], BF16)     # den rows at partitions {0,32,64,96}
        onesp_sb = sing.tile([128, 128], BF16)
        zb_sb = sing.tile([128, 1], F32)            # zero bias for activations

        nc.vector.memset(zb_sb[:], 0.0)
        nc.vector.memset(onesp_sb[:], 1.0)
        nc.vector.memset(
            v0_sb[:].rearrange("p (c w) -> p c w", w=VW)[:, :, HD : HD + 1], 1.0
        )
        nc.vector.memset(
            v1_sb[:].rearrange("p (c w) -> p c w", w=VW)[:, :, HD : HD + 1], 1.0
        )

        # ---- input DMAs (weights first; xT t-slice-major) ---------------
        def load_w(w_sb, w_dram):
            nc.sync.dma_start(
                out=w_sb[:].rearrange("p (c e) -> p c e", c=KD),
                in_=w_dram.rearrange("(c p) e -> p c e", p=128),
            )

        def load_xt(tb):
            # two DMA calls so the transfer spreads over two queues
            for h in range(2):
                c0, c1 = h * (KD // 2), (h + 1) * (KD // 2)
                nc.sync.dma_start(
                    out=xT_sb[:].rearrange("p (c t) -> p c t", c=KD)[
                        :, c0:c1, tb * TQ : (tb + 1) * TQ
                    ],
                    in_=xT[c0 * 128 : c1 * 128, tb * TQ : (tb + 1) * TQ].rearrange(
                        "(c p) t -> p c t", p=128
                    ),
                )

        load_xt(0)
        load_w(wq_sb, wqT)
        load_w(wk_sb, wkT)
        load_w(wv_sb, wvT)
        nc.scalar.dma_start(out=tri_sb[:], in_=tri)
        load_xt(1)

        def load_wp():
            nc.scalar.dma_start(
                out=wp_sb[:].rearrange("p (c e) -> p c e", c=KD),
                in_=wpT.rearrange("(c p) e -> p c e", p=128),
            )

        # ---- QKV helpers (emitted lazily as PE filler) -------------------
        def emit_q(tb):
            ts = tb * TQ
            psq = psS.tile([128, 1024], F32, tag="ps", name=f"psq{tb}")
            for kc in range(KD):
                nc.tensor.matmul(
                    out=psq[:, 0:TQ],
                    lhsT=wq_sb[:, kc * E : (kc + 1) * E],
                    rhs=xT_sb[:, kc * T + ts : kc * T + ts + TQ],
                    start=(kc == 0),
                    stop=(kc == KD - 1),
                )
            nc.vector.tensor_copy(out=qT_sb[:, ts : ts + TQ], in_=psq[:, 0:TQ])

        def emit_k(tb):
            ts = tb * TQ
            psk = psS.tile([128, 1024], F32, tag="ps", name=f"psk{tb}")
            for kc in range(KD):
                nc.tensor.matmul(
                    out=psk[:, 0:TQ],
                    lhsT=wk_sb[:, kc * E : (kc + 1) * E],
                    rhs=xT_sb[:, kc * T + ts : kc * T + ts + TQ],
                    start=(kc == 0),
                    stop=(kc == KD - 1),
                )
            nc.vector.tensor_copy(out=kT_sb[:, ts : ts + TQ], in_=psk[:, 0:TQ])

        def emit_v(ci):
            psv = psS.tile([128, 1024], F32, tag="ps", name=f"psv{ci}")
            for kc in range(KD):
                nc.tensor.matmul(
                    out=psv[:, 0:E],
                    lhsT=xT_sb[:, kc * T + ci * CK : kc * T + (ci + 1) * CK],
                    rhs=wv_sb[:, kc * E : (kc + 1) * E],
                    start=(kc == 0),
                    stop=(kc == KD - 1),
                )
            nc.vector.tensor_copy(
                out=v0_sb[:, ci * VW : ci * VW + HD], in_=psv[:, 0:HD]
            )
            nc.vector.tensor_copy(
                out=v1_sb[:, ci * VW : ci * VW + HD], in_=psv[:, HD:E]
            )

        # ---- exchange buffers: pairs 0-2 (shard s = 128 rows of the pair);
        # the last pair goes as two per-block exchanges (shard s = 64 rows)
        # so block 6's hides under block 7 and only block 7's is exposed.
        send_t = [dram.tile([NCORES, 128, 128], BF16, name=f"snd{k}") for k in range(NP - 1)]
        recv_t = [dram.tile([NCORES, 128, 128], BF16, name=f"rcv{k}") for k in range(NP - 1)]
        send_b = {b: dram.tile([NCORES, 128, 64], BF16, name=f"sndb{b}") for b in (6, 7)}
        recv_b = {b: dram.tile([NCORES, 128, 64], BF16, name=f"rcvb{b}") for b in (6, 7)}
        wup_s = dram.tile([NCORES, 128, 8], BF16, name="wups")
        wup_r = dram.tile([NCORES, 128, 8], BF16, name="wupr")

        def _a2a(src, dst):
            nc.gpsimd.collective_compute(
                "AllToAll",
                mybir.AluOpType.bypass,
                replica_groups=[list(range(NCORES))],
                ins=[src[:].opt()],
                outs=[dst[:].opt()],
            )

        def emit_a2a(k):
            _a2a(send_t[k], recv_t[k])

        # warmup collective: absorbs the CC entry-barrier setup cost early
        _a2a(wup_s, wup_r)

        def emit_norm_send(b, ylast=None, tail=False):
            # GpSimd broadcasts the denominator rows across partitions, then
            # DVE takes 1/x and one multiply; ship the block into its send
            # buffer (pair shards 0-3/4-7, or 8x64-row shards for blocks 6/7).
            rbb = psS.tile([128, TQ], F32, tag="ps", name=f"rbb{b}")
            for h in range(2):
                i = 2 * b + h
                pr = (i % 4) * 32
                cr = (i // 4) * TQ
                nc.tensor.matmul(
                    out=rbb[h * HD : (h + 1) * HD, :],
                    lhsT=onesp_sb[pr : pr + 1, 0:HD],
                    rhs=dsp_sb[pr : pr + 1, cr : cr + TQ],
                    start=True,
                    stop=True,
                    tile_position=(pr, h * HD),
                )
            rq = ynp.tile([128, TQ], F32, tag="rq", name=f"rq{b}")
            nc.vector.reciprocal_approx_fast(out=rq[:], in_=rbb[:])
            yn = ynp.tile([128, TQ], BF16, tag="yn", name=f"yn{b}")
            if ylast is None:
                nc.vector.tensor_mul(yn[:], y_sb[:, b * TQ : (b + 1) * TQ], rq[:])
            else:
                # tail block: normalize straight from the PSUM accumulators,
                # heads on different engines so the muls overlap
                y0t, y1t = ylast
                nc.vector.tensor_mul(yn[0:HD, :], y0t[0:HD, :], rq[0:HD, :])
                nc.vector.tensor_mul(yn[HD:128, :], y1t[0:HD, :], rq[HD:128, :])
            if b >= NB - 2:
                nc.sync.dma_start(
                    out=send_b[b][:].rearrange("s p q -> p s q"),
                    in_=yn[:].rearrange("p (s q) -> p s q", s=8),
                )
            else:
                k = b // 2
                s0 = (b % 2) * 4
                nc.sync.dma_start(
                    out=send_t[k][:].rearrange("s p q -> p s q")[:, s0 : s0 + 4, :],
                    in_=yn[:].rearrange("p (s q) -> p s q", s=4),
                )

        ybtiles = {}

        def emit_recv(k):
            # recv shard j = head-dims d in [128j, 128j+128) for my 128 rows
            yb = ybp.tile([128, NCORES * 128], BF16, tag="yb", name=f"yb{k}")
            nc.sync.dma_start(
                out=yb[:].rearrange("p (s q) -> p s q", s=NCORES),
                in_=recv_t[k][:].rearrange("s p q -> p s q"),
            )
            ybtiles[k] = yb

        def emit_projmm(k):
            yb = ybtiles[k]
            po = psS.tile([128, 1024], F32, tag="ps", name=f"po{k}")
            for nh in range(2):
                for j in range(NCORES):
                    nc.tensor.matmul(
                        out=po[:, nh * 512 : (nh + 1) * 512],
                        lhsT=yb[:, j * 128 : (j + 1) * 128],
                        rhs=wp_sb[:, j * D + nh * 512 : j * D + (nh + 1) * 512],
                        start=(j == 0),
                        stop=(j == NCORES - 1),
                    )
            o_sb = osb.tile([128, 1024], F32, tag="o", name=f"osb{k}")
            nc.vector.tensor_copy(out=o_sb[:], in_=po[:])
            nc.sync.dma_start(out=out[k * 128 : (k + 1) * 128, :], in_=o_sb[:])

        # last pair: per-block recv + half-height (M=64) projections so the
        # block-6 half runs hidden under block 7's attention.
        def emit_recv_b(b):
            yb = ybp.tile([128, NCORES * 64], BF16, tag="ybh", name=f"ybb{b}")
            nc.sync.dma_start(
                out=yb[:].rearrange("p (s q) -> p s q", s=NCORES),
                in_=recv_b[b][:].rearrange("s p q -> p s q"),
            )
            ybtiles[b + 100] = yb

        o3_sb = sing.tile([128, 1024], F32)

        def emit_projmm_b(b):
            yb = ybtiles[b + 100]
            po = psS.tile([128, 1024], F32, tag="ps", name=f"pob{b}")
            r0 = (b - 6) * 64
            for nh in range(2):
                for j in range(NCORES):
                    nc.tensor.matmul(
                        out=po[r0 : r0 + 64, nh * 512 : (nh + 1) * 512],
                        lhsT=yb[:, j * 64 : (j + 1) * 64],
                        rhs=wp_sb[:, j * D + nh * 512 : j * D + (nh + 1) * 512],
                        start=(j == 0),
                        stop=(j == NCORES - 1),
                    )
            nc.vector.tensor_copy(out=o3_sb[r0 : r0 + 64, :], in_=po[r0 : r0 + 64, :])

        # ---- flat attention pipeline -------------------------------------
        sched = [(b, sc) for b in range(NB) for sc in range(2 * (b + 1))]
        ytiles = {}

        def get_y(b):
            if b not in ytiles:
                ytiles[b] = (
                    psY.tile([VW, TQ], F32, tag="py", name=f"y0_{b}"),
                    psY.tile([VW, TQ], F32, tag="py", name=f"y1_{b}"),
                )
            return ytiles[b]

        def emit_s(b, sc):
            # two chunk tiles, each holding both heads (h0 cols 0:512,
            # h1 cols 512:1024); heads co-issue on PE row tiles.
            ts = b * TQ
            tiles = []
            for j in range(2):
                ci = 2 * sc + j
                off = ci * CK - ts
                qlo = max(off, 0)
                s = psS.tile([128, 1024], F32, tag="ps", name=f"s{b}_{sc}_{j}")
                nc.tensor.matmul(
                    out=s[:, qlo:TQ],
                    lhsT=kT_sb[0:HD, ci * CK : (ci + 1) * CK],
                    rhs=qT_sb[0:HD, ts + qlo : ts + TQ],
                    start=True,
                    stop=True,
                    tile_position=(0, 0),
                )
                nc.tensor.matmul(
                    out=s[:, TQ + qlo : 2 * TQ],
                    lhsT=kT_sb[HD:128, ci * CK : (ci + 1) * CK],
                    rhs=qT_sb[HD:128, ts + qlo : ts + TQ],
                    start=True,
                    stop=True,
                    tile_position=(64, 0),
                )
                tiles.append(s)
            return tiles

        def emit_p(b, sc, stiles):
            ts = b * TQ
            ptiles = []
            for j in range(2):
                ci = 2 * sc + j
                off = ci * CK - ts
                qlo = max(off, 0)
                s = stiles[j]
                p = pwork.tile([128, 1024], BF16, tag="pt", name=f"p{b}_{sc}_{j}")
                if qlo == 0:
                    ranges = [(0, 1024)]
                elif qlo <= CK:
                    # cols [TQ, TQ+qlo) hold garbage but are never consumed;
                    # one instr beats two for a small overlap
                    ranges = [(qlo, 2 * TQ)]
                else:
                    ranges = [(qlo, TQ), (TQ + qlo, 2 * TQ)]
                for (lo, hi) in ranges:
                    nc.scalar.activation(
                        out=p[:, lo:hi], in_=s[:, lo:hi],
                        func=mybir.ActivationFunctionType.Exp, bias=zb_sb[:],
                    )
                if off >= 0:
                    nc.vector.tensor_mul(
                        p[:, off : off + CK], p[:, off : off + CK], tri_sb[:]
                    )
                    nc.vector.tensor_mul(
                        p[:, TQ + off : TQ + off + CK],
                        p[:, TQ + off : TQ + off + CK],
                        tri_sb[:],
                    )
                ptiles.append(p)
            return ptiles

        def emit_pv(b, sc, ptiles):
            ts = b * TQ
            nchunks = 4 * (b + 1)
            y0t, y1t = get_y(b)
            for j in range(2):
                ci = 2 * sc + j
                off = ci * CK - ts
                qlo = max(off, 0)
                p = ptiles[j]
                nc.tensor.matmul(
                    out=y0t[:, qlo:TQ],
                    lhsT=v0_sb[:, ci * VW : (ci + 1) * VW],
                    rhs=p[:, qlo:TQ],
                    start=(ci == 0),
                    stop=(ci == nchunks - 1),
                )
                nc.tensor.matmul(
                    out=y1t[:, qlo:TQ],
                    lhsT=v1_sb[:, ci * VW : (ci + 1) * VW],
                    rhs=p[:, TQ + qlo : 2 * TQ],
                    start=(ci == 0),
                    stop=(ci == nchunks - 1),
                )

        def emit_block_end(b):
            # stash unnormalized y + denominator rows; block 7 keeps its
            # PSUM accumulators live for the tail's direct normalization.
            y0t, y1t = ytiles[b]
            for h, yy in ((0, y0t), (1, y1t)):
                i = 2 * b + h
                dst = dsp_sb[
                    (i % 4) * 32 : (i % 4) * 32 + 1,
                    (i // 4) * TQ : (i // 4 + 1) * TQ,
                ]
                if b == NB - 1 and h == 1:
                    nc.scalar.copy(out=dst, in_=yy[HD : HD + 1, :])
                else:
                    nc.vector.tensor_copy(out=dst, in_=yy[HD : HD + 1, :])
            if b < NB - 1:
                ts = b * TQ
                nc.vector.tensor_copy(out=y_sb[0:HD, ts : ts + TQ], in_=y0t[0:HD, :])
                nc.vector.tensor_copy(out=y_sb[HD:128, ts : ts + TQ], in_=y1t[0:HD, :])

        # pre/post hooks per (b, sc) slot
        hooks = {}

        def add_hook(b, sc, fn):
            hooks.setdefault((b, sc), []).append(fn)

        for b in range(1, NB):
            add_hook(b, 0, lambda bb=b: emit_norm_send(bb - 1))
            if b % 2 == 0 and b < NB - 1:
                add_hook(b, 0, lambda kk=b // 2 - 1: emit_a2a(kk))
        add_hook(NB - 1, 0, lambda: _a2a(send_b[6], recv_b[6]))
        add_hook(NB - 1, 10, lambda: emit_recv_b(6))
        add_hook(NB - 1, 12, lambda: emit_projmm_b(6))
        add_hook(
            NB - 1, 14,
            lambda: nc.sync.dma_start(
                out=out[(NP - 1) * 128 : (NP - 1) * 128 + 64, :], in_=o3_sb[0:64, :]
            ),
        )
        add_hook(3, 2, lambda: emit_recv(0))
        add_hook(4, 2, lambda: emit_projmm(0))
        add_hook(5, 2, lambda: emit_recv(1))
        add_hook(6, 2, lambda: emit_projmm(1))
        add_hook(7, 2, lambda: emit_recv(2))
        add_hook(7, 8, lambda: emit_projmm(2))
        # deferred input loads: xT block b arrives well ahead of first use
        for b in range(2, NB):
            add_hook(max(b - 3, 0), 1, lambda tb=b: load_xt(tb))
        add_hook(1, 1, load_wp)

        # QKV filler items per block, clustered into two groups per block to
        # minimize PE mode-switch drains at filler boundaries
        # QKV filler items per block, spread over its sc slots
        for b in range(NB - 1):
            items = [lambda tb=b + 1: emit_q(tb)]
            if b % 2 == 1 and b + 2 < NB:
                items.append(lambda tb=b + 1: emit_k(tb))
                items.append(lambda tb=b + 2: emit_k(tb))
            items += [lambda ci=c: emit_v(ci) for c in range(4 * b + 4, 4 * b + 8)]
            nsc = 2 * (b + 1)
            for idx, fn in enumerate(items):
                add_hook(b, (idx * nsc) // len(items), fn)

        # prologue
        emit_q(0)
        emit_k(0)
        emit_k(1)
        for ci in range(4):
            emit_v(ci)

        s_next = emit_s(*sched[0])
        p_prev = None
        prev_item = None
        for i, (b, sc) in enumerate(sched):
            cur_s = s_next
            p_cur = emit_p(b, sc, cur_s)
            if i + 1 < len(sched):
                s_next = emit_s(*sched[i + 1])
            if prev_item is not None:
                pb, psc = prev_item
                emit_pv(pb, psc, p_prev)
                if psc == 2 * (pb + 1) - 1:
                    emit_block_end(pb)
            for fn in hooks.get((b, sc), []):
                fn()
            p_prev, prev_item = p_cur, (b, sc)
        emit_pv(*prev_item, p_prev)
        emit_block_end(NB - 1)

        # ---- tail: last block's norm, exchange, projection ---------------
        emit_norm_send(NB - 1, ylast=ytiles[NB - 1])
        _a2a(send_b[7], recv_b[7])
        emit_recv_b(7)
        emit_projmm_b(7)
        nc.sync.dma_start(
            out=out[(NP - 1) * 128 + 64 : NP * 128, :], in_=o3_sb[64:128, :]
        )

    nc.compile()
    return nc


def _inputs(x, w_attn, w_proj):
    x = np.asarray(x, dtype=np.float32).reshape(T, D)
    w_attn = np.asarray(w_attn, dtype=np.float32)
    w_proj = np.asarray(w_proj, dtype=np.float32)

    xT_np = np.ascontiguousarray(x.T).astype(NPBF16)
    wpT_np = np.ascontiguousarray(w_proj.T).astype(NPBF16)
    scale = 1.0 / math.sqrt(HD)
    tri_np = np.triu(np.ones((128, 128), dtype=np.float32)).astype(NPBF16)

    in_maps = []
    for core in range(NCORES):
        r0 = core * E
        in_maps.append(
            {
                "xT": xT_np,
                "wqT": np.ascontiguousarray((w_attn[r0 : r0 + E, :] * scale).T).astype(
                    NPBF16
                ),
                "wkT": np.ascontiguousarray(w_attn[D + r0 : D + r0 + E, :].T).astype(
                    NPBF16
                ),
                "wvT": np.ascontiguousarray(
                    w_attn[2 * D + r0 : 2 * D + r0 + E, :].T
                ).astype(NPBF16),
                "wpT": wpT_np,
                "tri": tri_np,
            }
        )
    return in_maps


def kernel(x, w_attn, w_proj, _trace=False):
    if "nc" not in _CACHE:
        _CACHE["nc"] = _build()
    nc = _CACHE["nc"]
    in_maps = _inputs(x, w_attn, w_proj)
    res = run_bass_kernel_spmd(
        nc, in_maps, core_ids=list(range(NCORES)), trace=_trace
    )
    _CACHE["last_result"] = res
    full = np.empty((T, D), dtype=np.float32)
    for c in range(NCORES):
        o = res.results[c]["out"]  # [NP*128, D]
        for k in range(NP - 1):
            blk = 2 * k + (1 if c >= 4 else 0)
            q0 = (c % 4) * 128
            full[blk * TQ + q0 : blk * TQ + q0 + 128, :] = o[k * 128 : (k + 1) * 128, :]
        # last pair went as two per-block 64-row exchanges
        k = NP - 1
        q0 = c * 64
        full[6 * TQ + q0 : 6 * TQ + q0 + 64, :] = o[k * 128 : k * 128 + 64, :]
        full[7 * TQ + q0 : 7 * TQ + q0 + 64, :] = o[k * 128 + 64 : (k + 1) * 128, :]
    return full.reshape(B, T, D).astype(np.float32)


# revision 46
# speedup vs baseline: 1.0231x; 1.0058x over previous
"""Causal self-attention (B=1, T=4096, D=1024, H=16, HD=64) on 8 trn2 NeuronCores.

Sharding: tensor-parallel over heads (2 heads per core) for QKV + attention;
pipelined per-block-pair AllToAll re-shards to row-parallel for the output
projection (each core projects a 128-row slice of each 1024-row block pair).

Matmul layout notes (PE computes out = lhsT.T @ rhs, contraction on partitions):
 - host feeds x transposed (xT [D, T]) so QKV needs no on-chip transposes.
 - S^T tiles [tk, tq] are computed (not S) so the PV matmul can consume
   exp(S^T) directly as the moving operand with V in natural [tk, hd] layout.
 - S tiles are chunk-major with BOTH heads in one [128, 1024] tile
   (head0 at cols 0:512, head1 at 512:1024). The heads' QK matmuls use PE
   row tiles (0,0)/(64,0) and share one PSUM tile, so one exp releases
   both and the pair co-issues on the PE's 64-row tile mode.
 - a ones-column appended to V makes row 64 of the PV accumulator the
   softmax denominator (no extra reduction pass).
 - causal masking: QK/exp/PV streams are trimmed to the causally valid
   column range; the 128-wide diagonal band is zeroed in p by a DVE
   multiply with a precomputed triangle mask (no PE mask matmuls).
 - the whole attention stream runs as one flat software pipeline across
   block boundaries (exp(i) | S(i+1) | PV(i-1)) so the PE never drains at
   a block edge (keeps the PE p-state ramped).
 - softmax max-subtraction is skipped: scores are ~N(0,1) (|s| < ~10), and
   a constant shift cancels exactly in softmax, so exp is safe in fp32.
"""

import math
import sys
from contextlib import ExitStack

sys.path.insert(0, "/opt/trn_rl_repo")

import ml_dtypes
import numpy as np

import concourse.bass as bass  # noqa: F401  (bass types used via tile/bacc)
import concourse.mybir as mybir
import concourse.tile as tile
from concourse import bacc
from concourse.bass_utils import run_bass_kernel_spmd

B, T, D, H, HD = 1, 4096, 1024, 16, 64
NCORES = 8
HPC = H // NCORES          # heads per core = 2
E = HPC * HD               # per-core head width = 128
TQ = 512                   # tq block width
NB = T // TQ               # 8 tq blocks
CK = 128                   # tk chunk (partition dim of S^T tiles)
KD = D // 128              # 8 contraction chunks over D
NV = T // CK               # 32 tk chunks total
VW = HD + 1                # V tile width incl. ones column = 65
NP = NB // 2               # block pairs exchanged together = 4

BF16 = mybir.dt.bfloat16
F32 = mybir.dt.float32
NPBF16 = ml_dtypes.bfloat16

_CACHE = {}


def _build():
    nc = bacc.Bacc("TRN2", target_bir_lowering=False, debug=False, num_devices=NCORES)
    xT = nc.dram_tensor("xT", [D, T], BF16, kind="ExternalInput").ap()
    wqT = nc.dram_tensor("wqT", [D, E], BF16, kind="ExternalInput").ap()
    wkT = nc.dram_tensor("wkT", [D, E], BF16, kind="ExternalInput").ap()
    wvT = nc.dram_tensor("wvT", [D, E], BF16, kind="ExternalInput").ap()
    wpT = nc.dram_tensor("wpT", [D, D], BF16, kind="ExternalInput").ap()
    tri = nc.dram_tensor("tri", [128, 128], BF16, kind="ExternalInput").ap()
    out = nc.dram_tensor("out", [NP * 128, D], F32, kind="ExternalOutput").ap()

    with tile.TileContext(nc) as tc, ExitStack() as ctx:
        sing = ctx.enter_context(tc.tile_pool(name="sing", bufs=1))
        pwork = ctx.enter_context(tc.tile_pool(name="pwork", bufs=6))
        ynp = ctx.enter_context(tc.tile_pool(name="ynp", bufs=4))
        ybp = ctx.enter_context(tc.tile_pool(name="ybp", bufs=2))
        osb = ctx.enter_context(tc.tile_pool(name="osb", bufs=2))
        # PSUM (8 banks): psS 3 x [128,1024] (2 banks each) = 6 for S tiles,
        # QKV accumulators and proj; psY 2 x [65,512] (1 bank each) = 2.
        psS = ctx.enter_context(tc.tile_pool(name="psS", bufs=3, space="PSUM"))
        psY = ctx.enter_context(tc.tile_pool(name="psY", bufs=2, space="PSUM"))
        dram = ctx.enter_context(tc.tile_pool(name="dram", bufs=1, space="DRAM"))

        # ---- resident SBUF tensors -------------------------------------
        xT_sb = sing.tile([128, KD * T], BF16)      # d-chunk kc at cols [kc*T, (kc+1)*T)
        wq_sb = sing.tile([128, KD * E], BF16)
        wk_sb = sing.tile([128, KD * E], BF16)
        wv_sb = sing.tile([128, KD * E], BF16)
        wp_sb = sing.tile([128, KD * D], BF16)
        tri_sb = sing.tile([128, 128], BF16)        # tri[k,j] = 1{j>=k}
        qT_sb = sing.tile([128, T], BF16)           # rows 0:64 head0, 64:128 head1
        kT_sb = sing.tile([128, T], BF16)
        v0_sb = sing.tile([128, NV * VW], BF16)     # V head0 + ones col per chunk
        v1_sb = sing.tile([128, NV * VW], BF16)
        y_sb = sing.tile([128, T], F32)             # unnormalized y^T (both heads)
        dsp_sb = sing.tile([128, 4 * TQ], BF16)     # den rows at partitions {0,32,64,96}
        onesp_sb = sing.tile([128, 128], BF16)
        zb_sb = sing.tile([128, 1], F32)            # zero bias for activations

        nc.vector.memset(zb_sb[:], 0.0)
        nc.vector.memset(onesp_sb[:], 1.0)
        nc.vector.memset(
            v0_sb[:].rearrange("p (c w) -> p c w", w=VW)[:, :, HD : HD + 1], 1.0
        )
        nc.vector.memset(
            v1_sb[:].rearrange("p (c w) -> p c w", w=VW)[:, :, HD : HD + 1], 1.0
        )

        # ---- input DMAs (weights first; xT t-slice-major) ---------------
        def load_w(w_sb, w_dram):
            nc.sync.dma_start(
                out=w_sb[:].rearrange("p (c e) -> p c e", c=KD),
                in_=w_dram.rearrange("(c p) e -> p c e", p=128),
            )

        def load_xt(tb):
            # four DMA calls so the transfer spreads over four queues
            for h in range(4):
                c0, c1 = h * (KD // 4), (h + 1) * (KD // 4)
                nc.sync.dma_start(
                    out=xT_sb[:].rearrange("p (c t) -> p c t", c=KD)[
                        :, c0:c1, tb * TQ : (tb + 1) * TQ
                    ],
                    in_=xT[c0 * 128 : c1 * 128, tb * TQ : (tb + 1) * TQ].rearrange(
                        "(c p) t -> p c t", p=128
                    ),
                )

        load_xt(0)
        load_w(wq_sb, wqT)
        load_w(wk_sb, wkT)
        load_w(wv_sb, wvT)
        nc.scalar.dma_start(out=tri_sb[:], in_=tri)
        load_xt(1)

        def load_wp():
            nc.scalar.dma_start(
                out=wp_sb[:].rearrange("p (c e) -> p c e", c=KD),
                in_=wpT.rearrange("(c p) e -> p c e", p=128),
            )

        # ---- QKV helpers (emitted lazily as PE filler) -------------------
        def emit_q(tb):
            ts = tb * TQ
            psq = psS.tile([128, 1024], F32, tag="ps", name=f"psq{tb}")
            for kc in range(KD):
                nc.tensor.matmul(
                    out=psq[:, 0:TQ],
                    lhsT=wq_sb[:, kc * E : (kc + 1) * E],
                    rhs=xT_sb[:, kc * T + ts : kc * T + ts + TQ],
                    start=(kc == 0),
                    stop=(kc == KD - 1),
                )
            nc.vector.tensor_copy(out=qT_sb[:, ts : ts + TQ], in_=psq[:, 0:TQ])

        def emit_k(tb):
            ts = tb * TQ
            psk = psS.tile([128, 1024], F32, tag="ps", name=f"psk{tb}")
            for kc in range(KD):
                nc.tensor.matmul(
                    out=psk[:, 0:TQ],
                    lhsT=wk_sb[:, kc * E : (kc + 1) * E],
                    rhs=xT_sb[:, kc * T + ts : kc * T + ts + TQ],
                    start=(kc == 0),
                    stop=(kc == KD - 1),
                )
            nc.vector.tensor_copy(out=kT_sb[:, ts : ts + TQ], in_=psk[:, 0:TQ])

        def emit_v(ci):
            psv = psS.tile([128, 1024], F32, tag="ps", name=f"psv{ci}")
            for kc in range(KD):
                nc.tensor.matmul(
                    out=psv[:, 0:E],
                    lhsT=xT_sb[:, kc * T + ci * CK : kc * T + (ci + 1) * CK],
                    rhs=wv_sb[:, kc * E : (kc + 1) * E],
                    start=(kc == 0),
                    stop=(kc == KD - 1),
                )
            nc.vector.tensor_copy(
                out=v0_sb[:, ci * VW : ci * VW + HD], in_=psv[:, 0:HD]
            )
            nc.vector.tensor_copy(
                out=v1_sb[:, ci * VW : ci * VW + HD], in_=psv[:, HD:E]
            )

        # ---- exchange buffers: pairs 0-2 (shard s = 128 rows of the pair);
        # the last pair goes as two per-block exchanges (shard s = 64 rows)
        # so block 6's hides under block 7 and only block 7's is exposed.
        send_t = [dram.tile([NCORES, 128, 128], BF16, name=f"snd{k}") for k in range(NP - 1)]
        recv_t = [dram.tile([NCORES, 128, 128], BF16, name=f"rcv{k}") for k in range(NP - 1)]
        send_b = {b: dram.tile([NCORES, 128, 64], BF16, name=f"sndb{b}") for b in (6, 7)}
        recv_b = {b: dram.tile([NCORES, 128, 64], BF16, name=f"rcvb{b}") for b in (6, 7)}
        wup_s = dram.tile([NCORES, 128, 8], BF16, name="wups")
        wup_r = dram.tile([NCORES, 128, 8], BF16, name="wupr")

        def _a2a(src, dst):
            nc.gpsimd.collective_compute(
                "AllToAll",
                mybir.AluOpType.bypass,
                replica_groups=[list(range(NCORES))],
                ins=[src[:].opt()],
                outs=[dst[:].opt()],
            )

        def emit_a2a(k):
            _a2a(send_t[k], recv_t[k])

        # warmup collective: absorbs the CC entry-barrier setup cost early
        _a2a(wup_s, wup_r)

        def emit_norm_send(b, ylast=None, tail=False):
            # GpSimd broadcasts the denominator rows across partitions, then
            # DVE takes 1/x and one multiply; ship the block into its send
            # buffer (pair shards 0-3/4-7, or 8x64-row shards for blocks 6/7).
            rbb = psS.tile([128, TQ], F32, tag="ps", name=f"rbb{b}")
            for h in range(2):
                i = 2 * b + h
                pr = (i % 4) * 32
                cr = (i // 4) * TQ
                nc.tensor.matmul(
                    out=rbb[h * HD : (h + 1) * HD, :],
                    lhsT=onesp_sb[pr : pr + 1, 0:HD],
                    rhs=dsp_sb[pr : pr + 1, cr : cr + TQ],
                    start=True,
                    stop=True,
                    tile_position=(pr, h * HD),
                )
            rq = ynp.tile([128, TQ], F32, tag="rq", name=f"rq{b}")
            nc.vector.reciprocal_approx_fast(out=rq[:], in_=rbb[:])
            yn = ynp.tile([128, TQ], BF16, tag="yn", name=f"yn{b}")
            if ylast is None:
                nc.vector.tensor_mul(yn[:], y_sb[:, b * TQ : (b + 1) * TQ], rq[:])
            else:
                # tail block: normalize straight from the PSUM accumulators,
                # heads on different engines so the muls overlap
                y0t, y1t = ylast
                nc.vector.tensor_mul(yn[0:HD, :], y0t[0:HD, :], rq[0:HD, :])
                nc.vector.tensor_mul(yn[HD:128, :], y1t[0:HD, :], rq[HD:128, :])
            if b >= NB - 2:
                nc.sync.dma_start(
                    out=send_b[b][:].rearrange("s p q -> p s q"),
                    in_=yn[:].rearrange("p (s q) -> p s q", s=8),
                )
            else:
                k = b // 2
                s0 = (b % 2) * 4
                nc.sync.dma_start(
                    out=send_t[k][:].rearrange("s p q -> p s q")[:, s0 : s0 + 4, :],
                    in_=yn[:].rearrange("p (s q) -> p s q", s=4),
                )

        ybtiles = {}

        def emit_recv(k):
            # recv shard j = head-dims d in [128j, 128j+128) for my 128 rows
            yb = ybp.tile([128, NCORES * 128], BF16, tag="yb", name=f"yb{k}")
            nc.sync.dma_start(
                out=yb[:].rearrange("p (s q) -> p s q", s=NCORES),
                in_=recv_t[k][:].rearrange("s p q -> p s q"),
            )
            ybtiles[k] = yb

        def emit_projmm(k):
            yb = ybtiles[k]
            po = psS.tile([128, 1024], F32, tag="ps", name=f"po{k}")
            for nh in range(2):
                for j in range(NCORES):
                    nc.tensor.matmul(
                        out=po[:, nh * 512 : (nh + 1) * 512],
                        lhsT=yb[:, j * 128 : (j + 1) * 128],
                        rhs=wp_sb[:, j * D + nh * 512 : j * D + (nh + 1) * 512],
                        start=(j == 0),
                        stop=(j == NCORES - 1),
                    )
            o_sb = osb.tile([128, 1024], F32, tag="o", name=f"osb{k}")
            nc.vector.tensor_copy(out=o_sb[:], in_=po[:])
            nc.sync.dma_start(out=out[k * 128 : (k + 1) * 128, :], in_=o_sb[:])

        # last pair: per-block recv + half-height (M=64) projections so the
        # block-6 half runs hidden under block 7's attention.
        def emit_recv_b(b):
            yb = ybp.tile([128, NCORES * 64], BF16, tag="ybh", name=f"ybb{b}")
            nc.sync.dma_start(
                out=yb[:].rearrange("p (s q) -> p s q", s=NCORES),
                in_=recv_b[b][:].rearrange("s p q -> p s q"),
            )
            ybtiles[b + 100] = yb

        o3_sb = sing.tile([128, 1024], F32)

        def emit_projmm_b(b):
            yb = ybtiles[b + 100]
            po = psS.tile([128, 1024], F32, tag="ps", name=f"pob{b}")
            r0 = (b - 6) * 64
            for nh in range(2):
                for j in range(NCORES):
                    nc.tensor.matmul(
                        out=po[r0 : r0 + 64, nh * 512 : (nh + 1) * 512],
                        lhsT=yb[:, j * 64 : (j + 1) * 64],
                        rhs=wp_sb[:, j * D + nh * 512 : j * D + (nh + 1) * 512],
                        start=(j == 0),
                        stop=(j == NCORES - 1),
                    )
            nc.vector.tensor_copy(out=o3_sb[r0 : r0 + 64, :], in_=po[r0 : r0 + 64, :])

        # ---- flat attention pipeline -------------------------------------
        sched = [(b, sc) for b in range(NB) for sc in range(2 * (b + 1))]
        ytiles = {}

        def get_y(b):
            if b not in ytiles:
                ytiles[b] = (
                    psY.tile([VW, TQ], F32, tag="py", name=f"y0_{b}"),
                    psY.tile([VW, TQ], F32, tag="py", name=f"y1_{b}"),
                )
            return ytiles[b]

        def emit_s(b, sc):
            # two chunk tiles, each holding both heads (h0 cols 0:512,
            # h1 cols 512:1024); heads co-issue on PE row tiles.
            ts = b * TQ
            tiles = []
            for j in range(2):
                ci = 2 * sc + j
                off = ci * CK - ts
                qlo = max(off, 0)
                s = psS.tile([128, 1024], F32, tag="ps", name=f"s{b}_{sc}_{j}")
                nc.tensor.matmul(
                    out=s[:, qlo:TQ],
                    lhsT=kT_sb[0:HD, ci * CK : (ci + 1) * CK],
                    rhs=qT_sb[0:HD, ts + qlo : ts + TQ],
                    start=True,
                    stop=True,
                    tile_position=(0, 0),
                )
                nc.tensor.matmul(
                    out=s[:, TQ + qlo : 2 * TQ],
                    lhsT=kT_sb[HD:128, ci * CK : (ci + 1) * CK],
                    rhs=qT_sb[HD:128, ts + qlo : ts + TQ],
                    start=True,
                    stop=True,
                    tile_position=(64, 0),
                )
                tiles.append(s)
            return tiles

        def emit_p(b, sc, stiles):
            ts = b * TQ
            ptiles = []
            for j in range(2):
                ci = 2 * sc + j
                off = ci * CK - ts
                qlo = max(off, 0)
                s = stiles[j]
                p = pwork.tile([128, 1024], BF16, tag="pt", name=f"p{b}_{sc}_{j}")
                if qlo == 0:
                    ranges = [(0, 1024)]
                elif qlo <= CK:
                    # cols [TQ, TQ+qlo) hold garbage but are never consumed;
                    # one instr beats two for a small overlap
                    ranges = [(qlo, 2 * TQ)]
                else:
                    ranges = [(qlo, TQ), (TQ + qlo, 2 * TQ)]
                for (lo, hi) in ranges:
                    nc.scalar.activation(
                        out=p[:, lo:hi], in_=s[:, lo:hi],
                        func=mybir.ActivationFunctionType.Exp, bias=zb_sb[:],
                    )
                if off >= 0:
                    nc.vector.tensor_mul(
                        p[:, off : off + CK], p[:, off : off + CK], tri_sb[:]
                    )
                    nc.vector.tensor_mul(
                        p[:, TQ + off : TQ + off + CK],
                        p[:, TQ + off : TQ + off + CK],
                        tri_sb[:],
                    )
                ptiles.append(p)
            return ptiles

        def emit_pv(b, sc, ptiles):
            ts = b * TQ
            nchunks = 4 * (b + 1)
            y0t, y1t = get_y(b)
            for j in range(2):
                ci = 2 * sc + j
                off = ci * CK - ts
                qlo = max(off, 0)
                p = ptiles[j]
                nc.tensor.matmul(
                    out=y0t[:, qlo:TQ],
                    lhsT=v0_sb[:, ci * VW : (ci + 1) * VW],
                    rhs=p[:, qlo:TQ],
                    start=(ci == 0),
                    stop=(ci == nchunks - 1),
                )
                nc.tensor.matmul(
                    out=y1t[:, qlo:TQ],
                    lhsT=v1_sb[:, ci * VW : (ci + 1) * VW],
                    rhs=p[:, TQ + qlo : 2 * TQ],
                    start=(ci == 0),
                    stop=(ci == nchunks - 1),
                )

        def emit_block_end(b):
            # stash unnormalized y + denominator rows; block 7 keeps its
            # PSUM accumulators live for the tail's direct normalization.
            y0t, y1t = ytiles[b]
            for h, yy in ((0, y0t), (1, y1t)):
                i = 2 * b + h
                dst = dsp_sb[
                    (i % 4) * 32 : (i % 4) * 32 + 1,
                    (i // 4) * TQ : (i // 4 + 1) * TQ,
                ]
                if b == NB - 1 and h == 1:
                    nc.scalar.copy(out=dst, in_=yy[HD : HD + 1, :])
                else:
                    nc.vector.tensor_copy(out=dst, in_=yy[HD : HD + 1, :])
            if b < NB - 1:
                ts = b * TQ
                nc.vector.tensor_copy(out=y_sb[0:HD, ts : ts + TQ], in_=y0t[0:HD, :])
                nc.vector.tensor_copy(out=y_sb[HD:128, ts : ts + TQ], in_=y1t[0:HD, :])

        # pre/post hooks per (b, sc) slot
        hooks = {}

        def add_hook(b, sc, fn):
            hooks.setdefault((b, sc), []).append(fn)

        for b in range(1, NB):
            add_hook(b, 0, lambda bb=b: emit_norm_send(bb - 1))
            if b % 2 == 0 and b < NB - 1:
                add_hook(b, 0, lambda kk=b // 2 - 1: emit_a2a(kk))
        add_hook(NB - 1, 0, lambda: _a2a(send_b[6], recv_b[6]))
        add_hook(NB - 1, 10, lambda: emit_recv_b(6))
        add_hook(NB - 1, 12, lambda: emit_projmm_b(6))
        add_hook(
            NB - 1, 14,
            lambda: nc.sync.dma_start(
                out=out[(NP - 1) * 128 : (NP - 1) * 128 + 64, :], in_=o3_sb[0:64, :]
            ),
        )
        add_hook(3, 2, lambda: emit_recv(0))
        add_hook(4, 2, lambda: emit_projmm(0))
        add_hook(5, 2, lambda: emit_recv(1))
        add_hook(6, 2, lambda: emit_projmm(1))
        add_hook(7, 2, lambda: emit_recv(2))
        add_hook(7, 8, lambda: emit_projmm(2))
        # deferred input loads: xT block b arrives well ahead of first use
        for b in range(2, NB):
            add_hook(max(b - 3, 0), 1, lambda tb=b: load_xt(tb))
        add_hook(1, 1, load_wp)

        # QKV filler items per block, clustered into two groups per block to
        # minimize PE mode-switch drains at filler boundaries
        # QKV filler items per block, spread over its sc slots
        for b in range(NB - 1):
            items = [lambda tb=b + 1: emit_q(tb)]
            if b % 2 == 1 and b + 2 < NB:
                items.append(lambda tb=b + 1: emit_k(tb))
                items.append(lambda tb=b + 2: emit_k(tb))
            items += [lambda ci=c: emit_v(ci) for c in range(4 * b + 4, 4 * b + 8)]
            nsc = 2 * (b + 1)
            for idx, fn in enumerate(items):
                add_hook(b, (idx * nsc) // len(items), fn)

        # prologue
        emit_q(0)
        emit_k(0)
        emit_k(1)
        for ci in range(4):
            emit_v(ci)

        s_next = emit_s(*sched[0])
        p_prev = None
        prev_item = None
        for i, (b, sc) in enumerate(sched):
            cur_s = s_next
            p_cur = emit_p(b, sc, cur_s)
            if i + 1 < len(sched):
                s_next = emit_s(*sched[i + 1])
            if prev_item is not None:
                pb, psc = prev_item
                emit_pv(pb, psc, p_prev)
                if psc == 2 * (pb + 1) - 1:
                    emit_block_end(pb)
            for fn in hooks.get((b, sc), []):
                fn()
            p_prev, prev_item = p_cur, (b, sc)
        emit_pv(*prev_item, p_prev)
        emit_block_end(NB - 1)

        # ---- tail: last block's norm, exchange, projection ---------------
        emit_norm_send(NB - 1, ylast=ytiles[NB - 1])
        _a2a(send_b[7], recv_b[7])
        emit_recv_b(7)
        emit_projmm_b(7)
        nc.sync.dma_start(
            out=out[(NP - 1) * 128 + 64 : NP * 128, :], in_=o3_sb[64:128, :]
        )

    nc.compile()
    return nc


def _inputs(x, w_attn, w_proj):
    x = np.asarray(x, dtype=np.float32).reshape(T, D)
    w_attn = np.asarray(w_attn, dtype=np.float32)
    w_proj = np.asarray(w_proj, dtype=np.float32)

    xT_np = np.ascontiguousarray(x.T).astype(NPBF16)
    wpT_np = np.ascontiguousarray(w_proj.T).astype(NPBF16)
    scale = 1.0 / math.sqrt(HD)
    tri_np = np.triu(np.ones((128, 128), dtype=np.float32)).astype(NPBF16)

    in_maps = []
    for core in range(NCORES):
        r0 = core * E
        in_maps.append(
            {
                "xT": xT_np,
                "wqT": np.ascontiguousarray((w_attn[r0 : r0 + E, :] * scale).T).astype(
                    NPBF16
                ),
                "wkT": np.ascontiguousarray(w_attn[D + r0 : D + r0 + E, :].T).astype(
                    NPBF16
                ),
                "wvT": np.ascontiguousarray(
                    w_attn[2 * D + r0 : 2 * D + r0 + E, :].T
                ).astype(NPBF16),
                "wpT": wpT_np,
                "tri": tri_np,
            }
        )
    return in_maps


def kernel(x, w_attn, w_proj, _trace=False):
    if "nc" not in _CACHE:
        _CACHE["nc"] = _build()
    nc = _CACHE["nc"]
    in_maps = _inputs(x, w_attn, w_proj)
    res = run_bass_kernel_spmd(
        nc, in_maps, core_ids=list(range(NCORES)), trace=_trace
    )
    _CACHE["last_result"] = res
    full = np.empty((T, D), dtype=np.float32)
    for c in range(NCORES):
        o = res.results[c]["out"]  # [NP*128, D]
        for k in range(NP - 1):
            blk = 2 * k + (1 if c >= 4 else 0)
            q0 = (c % 4) * 128
            full[blk * TQ + q0 : blk * TQ + q0 + 128, :] = o[k * 128 : (k + 1) * 128, :]
        # last pair went as two per-block 64-row exchanges
        k = NP - 1
        q0 = c * 64
        full[6 * TQ + q0 : 6 * TQ + q0 + 64, :] = o[k * 128 : k * 128 + 64, :]
        full[7 * TQ + q0 : 7 * TQ + q0 + 64, :] = o[k * 128 + 64 : (k + 1) * 128, :]
    return full.reshape(B, T, D).astype(np.float32)
